# revision 3
# baseline (speedup 1.0000x reference)
"""Trainium2 Bass kernel for NeuroVPR Vanilla SNN (3-layer LIF, T=3).

Data-parallel over batch: B=16384 -> 2048 per core x 8 cores.

Math (per timestep, per layer): v = (v_prev + h)/2; s = (v>=1); v *= (1-s).
The LIF recurrence is homogeneous and the decay is a power of 2, so each
layer runs in a scaled basis u_t = 2^t * 2c * v_t (c = weight prescale,
32 for L1 / 16 for L2-L3, lifting weights out of fp8's subnormal range):
    u_t = m_{t-1} + 2^t * psum_t      (the *0.5 decay cancels)
    s_t = (u_t >= 2^t * 2c)
    m_t = u_t * (u_t < 2^t * 2c)
The 2^t factor rides the ScalarE extract's free `scale` field; thresholds
double each timestep (exact powers of 2). Spike decisions match the fp32
recurrence up to matmul quantization error.

All matmuls are fp8e4 perf_mode=DoubleRow (K=256/instr, warm issue rate
216 ns at N=512). Hidden-layer spikes live in a +/-1 (ScalarE Sign) or
+/-0.5 (VectorE is_ge/sub) basis: the next layer's ScalarE extract scale
absorbs the basis change (SSC) and the row-sum correction rides that
layer's bias column, precomputed on host from the quantized weights.
L1's bias rides a ones row appended to x (pad row D).

Schedule: t=0/1 run as half-batch passes (2 double-bank psum groups, k
inner); t=2 runs as FOUR quarter-passes of 512 columns so the L2(t2)/
L3(t2) chains pipeline against the remaining L1 matmul stream and only
the last quarter's short chain trails the final matmul. L2/L3 matmul
groups are hooked into the k-loops of later passes so the in-order PE
queue never waits on their PSUM-extract dependencies. L3(t2) needs no
ScalarE extract: u = 0.125*m3 + psum on VectorE, and the row-sum
correction folds into a per-partition threshold column (tensor_scalar
with AP scalar), so no ones-matmul. s1(t2) spikes use ScalarE Sign
(+/-1, SSC=4) to keep VectorE under budget in the endgame.
x half-tiles ([128, 2048] fp8 per k-slab per half-batch) are
DMA-prefetched one pass ahead; the initial prefetch is split across the
sync and scalar queues with w1's k=0 slab first on sync (the gpsimd
software queue is slow to start, ~28 GB/s; it only carries w2/w3/bias).
GpSimd compute is avoided entirely (measured ~8us per [128,512] op).
"""
import os
import numpy as np
import ml_dtypes

B, T, D = 16384, 3, 2752
DP = 2816          # D padded to 11*256
KD = DP // 256     # 11 DoubleRow contraction slabs
H, O = 256, 100
OP = 112           # O padded so the DoubleRow pair-stride is 16B-aligned
NCORES = 8
BC = B // NCORES   # 2048
NB = 512           # matmul free-dim block
WB = 1024          # half-pass LIF elementwise span (2 psum banks)
HB = BC // 2       # half-batch per L1 pass (1024)

SC1, SC2 = 32.0, 8.0    # weight prescale: L1; L2/L3 (+/-1 spike basis)
TH1, TH2 = 64.0, 32.0   # base thresholds (scaled x2 each timestep)
EPS = 0.0625            # tie-break so Sign(u - (th-EPS)) == +/-1 with s=1 at u==th

_compiled = None
last_results = None  # BassKernelResults of the most recent run (for profiling)


def _build():
    from contextlib import ExitStack
    import concourse.bass as bass
    import concourse.mybir as mybir
    import concourse.tile as tile
    from concourse import bacc

    f8 = mybir.dt.float8e4
    bf16 = mybir.dt.bfloat16
    f32 = mybir.dt.float32
    A = mybir.AluOpType
    DR = mybir.MatmulPerfMode.DoubleRow
    IDENT = mybir.ActivationFunctionType.Identity
    SIGN = mybir.ActivationFunctionType.Sign

    nc = bacc.Bacc("TRN2", target_bir_lowering=False, debug=False)
    x = nc.dram_tensor("x", [T, KD, 2, 128, HB * 2], f8, kind="ExternalInput").ap()
    w1 = nc.dram_tensor("w1", [128, KD * 2 * H], f8, kind="ExternalInput").ap()
    w2 = nc.dram_tensor("w2", [128, 2 * H], f8, kind="ExternalInput").ap()
    w3 = nc.dram_tensor("w3", [128, 2 * OP], f8, kind="ExternalInput").ap()
    bias = nc.dram_tensor("bias", [128, 22], f32, kind="ExternalInput").ap()
    out = nc.dram_tensor("out", [O, BC], f32, kind="ExternalOutput").ap()

    with tile.TileContext(nc) as tc, ExitStack() as ctx:
        wp = ctx.enter_context(tc.tile_pool(name="wp", bufs=1))
        xp = ctx.enter_context(tc.tile_pool(name="xp", bufs=24))
        pp1 = ctx.enter_context(tc.tile_pool(name="pp1", bufs=3, space="PSUM"))
        pp23 = ctx.enter_context(tc.tile_pool(name="pp23", bufs=1, space="PSUM"))
        sp = ctx.enter_context(tc.tile_pool(name="sp", bufs=1))
        tp = ctx.enter_context(tc.tile_pool(name="tp", bufs=6))

        # ---- ACT warmup first: fully host-data-independent ----
        wu = wp.tile([128, 8], bf16)
        wub = wp.tile([128, 1], f32)
        nc.vector.memset(wu[:, :], 0.0)
        nc.vector.memset(wub[:, :], 0.0)
        nc.scalar.activation(wu[:, 0:4], wu[:, 4:8], IDENT, bias=wub[:, 0:1])

        # ---- weights / bias loads ----
        # w1 k=0 slab rides the sync HW queue FIRST (gates the first matmul);
        # the rest of w1 rides scalar ahead of that queue's x tiles.
        w1t = wp.tile([128, KD * 2 * H], f8)
        nc.sync.dma_start(out=w1t[:, 0:512], in_=w1[:, 0:512])
        for c0, c1 in ((512, 2048), (2048, 4096), (4096, KD * 512)):
            nc.scalar.dma_start(out=w1t[:, c0:c1], in_=w1[:, c0:c1])
        w1o = w1t[:, :].rearrange("p (k two m) -> p k two m", k=KD, two=2)
        w2t = wp.tile([128, 2 * H], f8)
        nc.gpsimd.dma_start(out=w2t[:, :], in_=w2[:, :])
        w2o = w2t[:, :].rearrange("p (two m) -> p two m", two=2)
        w3t = wp.tile([128, 2 * OP], f8)
        nc.gpsimd.dma_start(out=w3t[:, :], in_=w3[:, :])
        w3o = w3t[:, :].rearrange("p (two m) -> p two m", two=2)
        bt = wp.tile([128, 22], f32)
        nc.gpsimd.dma_start(out=bt[:, :], in_=bias[:, :])
        # column layout (host fills): 0-5 beta1[t,h]; 6-11 beta2[t,h];
        # 12-14 beta3[t]; 15-17 -(2^t*TH1-EPS); 18-20 -(2^t*TH2-EPS);
        # 21 final L3 threshold 16 - rs3/2 - SC2*b3
        B1 = lambda t, h: bt[:, 2 * t + h: 2 * t + h + 1]
        B2 = lambda t, h: bt[:, 6 + 2 * t + h: 6 + 2 * t + h + 1]
        B3 = lambda t: bt[:, 12 + t: 13 + t]
        N1 = lambda t: bt[:, 15 + t: 16 + t]
        N2 = lambda t: bt[:, 18 + t: 19 + t]
        TH3C = bt[:, 21:22]

        # ---- persistent state (m = scaled membrane, written at t=0) ----
        m1 = [sp.tile([128, BC], bf16, tag=f"m1_{h}", name=f"m1_{h}")
              for h in range(2)]
        m2 = [sp.tile([128, BC], bf16, tag=f"m2_{h}", name=f"m2_{h}")
              for h in range(2)]
        m3 = sp.tile([128, BC], bf16, tag="m3")
        s1 = sp.tile([128, 2 * BC], f8, tag="s1")
        s2 = sp.tile([128, 2 * BC], f8, tag="s2")
        s1r = s1[:, :].rearrange("p (two n) -> p two n", two=2)
        s2r = s2[:, :].rearrange("p (two n) -> p two n", two=2)
        outsb = sp.tile([128, BC], f32, tag="outsb")

        xt = {}  # (t, k, half) -> x tile handle [128, 2*HB]

        def x_fetch(t, k, half, q=None):
            xt[t, k, half] = xp.tile([128, 2 * HB], f8, tag="x",
                                     name=f"x_{t}_{k}_{half}")
            (q or nc.sync).dma_start(out=xt[t, k, half][:, :],
                                     in_=x[t, k, half, :, :])

        SSC = [2.0, 2.0, 4.0]  # 2^t x (2 if spikes were +/-0.5 basis)

        def lif(ps, m_ap, s_ap, bcol, nthcol, th, t, P=128, sc=None):
            """Scaled-LIF on one [P, WB] psum span (t < T-1 paths only):
            ScalarE extract + bf16 VectorE ops; spike on ScalarE Sign at
            t=1 (DVE-heavy step), VectorE is_ge/sub otherwise."""
            hb = tp.tile([128, WB], bf16, tag="hb", name="hb")[:P, :]
            nc.scalar.activation(hb, ps, IDENT, bias=bcol[:P, :],
                                 scale=float(2 ** t) if sc is None else sc)
            if t == 0:
                u = hb
            else:
                u = tp.tile([128, WB], bf16, tag="u", name="u")[:P, :]
                nc.vector.tensor_tensor(u, m_ap, hb, A.add)
            if s_ap is not None:
                if t == 1:
                    nc.scalar.activation(s_ap, u, SIGN, bias=nthcol[:P, :])
                else:
                    nc.vector.tensor_scalar(s_ap, u, th * 2 ** t, 0.5,
                                            A.is_ge, A.subtract)
            nc.vector.scalar_tensor_tensor(m_ap, u, th * 2 ** t, u,
                                           A.is_lt, A.mult)

        def l1_pass(t, half, hooks=None):
            """One half-batch L1 pass (t < T-1): 2 double-bank psum groups,
            k inner. Prefetches the next pass's x tiles; `hooks[k]` emits
            interleaved L2/L3 work (their MMs slot into the PE stream)."""
            boff = half * HB
            ps = [pp1.tile([128, WB], f32, tag="ps1", name=f"ps1_{t}_{half}_{h}")
                  for h in range(2)]
            for k in range(KD):
                for fn in (hooks or {}).get(k, []):
                    fn()
                xr = xt[t, k, half][:, :].rearrange("p (two n) -> p two n", two=2)
                for h in range(2):
                    for b in range(2):
                        nc.tensor.matmul(
                            ps[h][:, b * NB:(b + 1) * NB],
                            w1o[:, k, :, h * 128:(h + 1) * 128],
                            xr[:, :, b * NB:(b + 1) * NB],
                            start=(k == 0), stop=(k == KD - 1), perf_mode=DR,
                            skip_group_check=True)
                if half == 0:
                    x_fetch(t, k, 1)
                else:
                    x_fetch(t + 1, k, 0)
            for h in range(2):
                bs = slice(boff, boff + WB)
                lif(ps[h][:, :], m1[h][:, bs],
                    s1[:, h * BC + boff: h * BC + boff + WB],
                    B1(t, h), N1(t), TH1, t)

        def l1_quarter(q, hooks=None):
            """One 512-column L1 quarter at t=T-1: k inner, 2 MMs per slab
            into the two banks of one [128,1024] psum tile. LIF runs on
            VectorE straight from PSUM (u = 0.25*m1 + psum); s1 spikes on
            ScalarE Sign (+/-1 basis -> SSC[2]=4). No m1 update needed."""
            half, b = q // 2, q % 2
            qs = slice(q * NB, (q + 1) * NB)
            ps = pp1.tile([128, WB], f32, tag="ps1", name=f"psq_{q}")
            for k in range(KD):
                for fn in (hooks or {}).get(k, []):
                    fn()
                xr = xt[T - 1, k, half][:, :].rearrange(
                    "p (two n) -> p two n", two=2)
                for h in range(2):
                    nc.tensor.matmul(
                        ps[:, h * NB:(h + 1) * NB],
                        w1o[:, k, :, h * 128:(h + 1) * 128],
                        xr[:, :, b * NB:(b + 1) * NB],
                        start=(k == 0), stop=(k == KD - 1), perf_mode=DR,
                        skip_group_check=True)
            for h in range(2):
                u = tp.tile([128, WB], bf16, tag="u", name="u")[:, 0:NB]
                nc.vector.scalar_tensor_tensor(u, m1[h][:, qs], 0.25,
                                               ps[:, h * NB:(h + 1) * NB],
                                               A.mult, A.add)
                nc.scalar.activation(s1[:, h * BC + q * NB:
                                        h * BC + (q + 1) * NB],
                                     u, SIGN, bias=N1(0))

        def l2_group(t, h, blk, pool, tag, ch=None):
            """L2 matmuls for one [128, 1024] span (blk in units of WB) or,
            with ch, one [128,512] quarter into ch's h-bank."""
            if ch is None:
                ps2 = pool.tile([128, WB], f32, tag=tag, name=f"ps2_{t}_{h}_{blk}")
                for b in range(2):
                    nc.tensor.matmul(
                        ps2[:, b * NB:(b + 1) * NB],
                        w2o[:, :, h * 128:(h + 1) * 128],
                        s1r[:, :, (2 * blk + b) * NB:(2 * blk + b + 1) * NB],
                        start=True, stop=True, perf_mode=DR,
                        skip_group_check=True)
                return ps2
            nc.tensor.matmul(ch[:, h * NB:(h + 1) * NB],
                             w2o[:, :, h * 128:(h + 1) * 128],
                             s1r[:, :, blk * NB:(blk + 1) * NB],
                             start=True, stop=True, perf_mode=DR,
                             skip_group_check=True)
            return ch

        def l2_one(t, h, bp):
            """Full-span L2 chain (t < T-1), psum from the shared ps23 ring."""
            bs = slice(bp * WB, (bp + 1) * WB)
            ps2 = l2_group(t, h, bp, pp23, "ps23")
            lif(ps2[:, :], m2[h][:, bs],
                s2[:, h * BC + bp * WB: h * BC + (bp + 1) * WB],
                B2(t, h), N2(t), TH2, t, sc=SSC[t])

        def l3_one(t, bp):
            """Full-span L3 chain (t < T-1): updates m3 only."""
            bs = slice(bp * WB, (bp + 1) * WB)
            ps3 = pp23.tile([128, WB], f32, tag="ps23", name=f"ps3_{t}_{bp}")
            for b in range(2):
                nc.tensor.matmul(ps3[:OP, b * NB:(b + 1) * NB], w3o[:, :, :],
                                 s2r[:, :, (2 * bp + b) * NB:(2 * bp + b + 1) * NB],
                                 start=True, stop=True, perf_mode=DR,
                                 skip_group_check=True)
            lif(ps3[:OP, :], m3[:OP, bs], None, B3(t), None, TH2, t,
                P=OP, sc=SSC[t])

        # ---- t=2 quarter chains ----
        cht = {}  # q -> shared L2/L3 chain psum tile

        def c_l2(q):
            """L2(t2) for quarter q: 2 MMs into one ps23 tile's banks, then
            ScalarE extract (+/-1 s1 basis -> scale 4) + VectorE add +
            VectorE is_ge/sub (+/-0.5 s2 basis)."""
            qs = slice(q * NB, (q + 1) * NB)
            ch = pp23.tile([128, WB], f32, tag="ps23", name=f"chq_{q}")
            cht[q] = ch
            for h in range(2):
                l2_group(2, h, q, None, None, ch=ch)
            for h in range(2):
                hb = tp.tile([128, WB], bf16, tag="hb", name="hb")[:, 0:NB]
                nc.scalar.activation(hb, ch[:, h * NB:(h + 1) * NB], IDENT,
                                     bias=B2(2, h), scale=SSC[2])
                u = tp.tile([128, WB], bf16, tag="u", name="u")[:, 0:NB]
                nc.vector.tensor_tensor(u, m2[h][:, qs], hb, A.add)
                nc.vector.tensor_scalar(s2[:, h * BC + q * NB:
                                           h * BC + (q + 1) * NB],
                                        u, TH2 * 4, 0.5, A.is_ge, A.subtract)

        def c_l3(q):
            """L3(t2) for quarter q: one MM reusing the chain tile's first
            bank, then u = 0.125*m3 + psum and is_ge against the
            per-partition threshold column (row-sum corr folded in)."""
            qs = slice(q * NB, (q + 1) * NB)
            ch = cht[q]
            nc.tensor.matmul(ch[:OP, 0:NB], w3o[:, :, :],
                             s2r[:, :, q * NB:(q + 1) * NB],
                             start=True, stop=True, perf_mode=DR,
                             skip_group_check=True)
            u = tp.tile([128, WB], bf16, tag="u", name="u")[:OP, 0:NB]
            nc.vector.scalar_tensor_tensor(u, m3[:OP, qs], 0.125,
                                           ch[:OP, 0:NB], A.mult, A.add)
            nc.vector.tensor_scalar(outsb[:OP, qs], u, TH3C[:OP, :], None,
                                    A.is_ge)
            (nc.sync if q % 2 == 0 else nc.scalar).dma_start(
                out=out[:, qs], in_=outsb[:O, qs])

        # ---- initial prefetch: sync gets w1 k0 + most x; scalar takes
        # w1's tail slabs then two late x tiles ----
        for k in (0, 1, 2, 3, 4, 5, 6, 8, 10):
            x_fetch(0, k, 0)
        x_fetch(0, 7, 0, nc.scalar)
        x_fetch(0, 9, 0, nc.scalar)

        l1_pass(0, 0)
        l1_pass(0, 1)
        l1_pass(1, 0, hooks={3: [lambda: l2_one(0, 0, 0)],
                             7: [lambda: l2_one(0, 1, 0)]})
        l1_pass(1, 1, hooks={1: [lambda: l2_one(0, 0, 1)],
                             4: [lambda: l2_one(0, 1, 1)],
                             8: [lambda: l3_one(0, 0)]})
        # t=2: four quarter passes; half1 x tiles fetched across q0/q1
        q0h = {1: [lambda: l3_one(0, 1)],
               5: [lambda: l2_one(1, 0, 0)],
               8: [lambda: l2_one(1, 1, 0)]}
        for k in range(KD):
            if k % 2 == 0:
                j = k // 2
                q0h.setdefault(k, []).append(
                    lambda j=j: x_fetch(2, j, 1))
        q1h = {3: [lambda: l3_one(1, 0)],
               6: [lambda: l2_one(1, 0, 1)],
               8: [lambda: l2_one(1, 1, 1)],
               10: [lambda: c_l2(0)]}
        for k in (0, 2, 4, 6, 8):
            j = 6 + k // 2
            if j < KD:
                q1h.setdefault(k, []).append(lambda j=j: x_fetch(2, j, 1))
        l1_quarter(0, hooks=q0h)
        l1_quarter(1, hooks=q1h)
        l1_quarter(2, hooks={3: [lambda: c_l3(0)],
                             5: [lambda: l3_one(1, 1)],
                             9: [lambda: c_l2(1)]})
        l1_quarter(3, hooks={2: [lambda: c_l3(1)],
                             7: [lambda: c_l2(2)]})
        c_l3(2)
        c_l2(3)
        c_l3(3)

    nc.compile()
    return nc


def kernel(dvs, W1, b1, W2, b2, W3, b3):
    global _compiled, last_results
    from concourse.bass_utils import run_bass_kernel_spmd

    if _compiled is None:
        _compiled = _build()
    nc = _compiled

    f8 = ml_dtypes.float8_e4m3

    def q8(a, scale):
        return np.clip(a * scale, -240.0, 240.0).astype(f8)

    # x: [B, T, D] -> fp8 [T, KD, 128, 2, B]  (d = k*256 + two*128 + p)
    x8 = q8(dvs, 1.0).transpose(1, 2, 0)          # [T, D, B]
    X = np.zeros((T, KD, 2, 128, B), dtype=f8)
    X.reshape(T, DP, B)[:, :D, :] = x8
    X.reshape(T, DP, B)[:, D, :] = f8(1.0)        # bias row (w1 row D = c1*b1)
    X = np.ascontiguousarray(X.transpose(0, 1, 3, 2, 4))  # [T, KD, 128, 2, B]

    # w1: [DP, H] scaled by SC1 -> [128, KD, 2, H]
    w1p = np.zeros((KD, 2, 128, H), dtype=f8)
    w1p.reshape(DP, H)[:D, :] = q8(W1.T, SC1)
    w1p.reshape(DP, H)[D, :] = q8(b1, SC1)
    w1p = np.ascontiguousarray(w1p.transpose(2, 0, 1, 3)).reshape(128, KD * 2 * H)
    # w2/w3 scaled by SC2 (+/-1 spike basis)
    w2q = q8(W2.T, SC2)                            # [H, H] j-major
    w2p = np.ascontiguousarray(
        w2q.reshape(2, 128, H).transpose(1, 0, 2)).reshape(128, 2 * H)
    w3q = np.zeros((H, OP), dtype=f8)
    w3q[:, :O] = q8(W3.T, SC2)
    w3p = np.ascontiguousarray(
        w3q.reshape(2, 128, OP).transpose(1, 0, 2)).reshape(128, 2 * OP)

    # bias/threshold columns; row-sum corrections use the quantized weights
    rs2 = w2q.astype(np.float64).sum(axis=0)       # [H]
    rs3 = w3q.astype(np.float64).sum(axis=0)       # [OP]
    bc = np.zeros((128, 22), dtype=np.float32)
    for t in range(T):
        p2 = float(2 ** t)
        for h in range(2):
            bc[:, 6 + 2 * t + h] = p2 * (rs2[h * 128:(h + 1) * 128]
                                         + 2 * SC2 * b2[h * 128:(h + 1) * 128])
        bc[:OP, 12 + t] = p2 * rs3
        bc[:O, 12 + t] += p2 * 2 * SC2 * b3
        bc[:, 15 + t] = -(p2 * TH1 - EPS)
        bc[:, 18 + t] = -(p2 * TH2 - EPS)
    # final L3 threshold column: s3 = (0.125*m3 + psum >= 16 - corr),
    # corr = rs3/2 + SC2*b3 (the +/-0.5 s2 basis row-sum correction)
    bc[:OP, 21] = 16.0 - rs3 / 2
    bc[:O, 21] -= SC2 * b3

    in_maps = []
    for c in range(NCORES):
        xc = X[:, :, :, :, c * BC:(c + 1) * BC]    # [T, KD, 128, 2, BC]
        xc = np.ascontiguousarray(
            xc.reshape(T, KD, 128, 2, 2, HB).transpose(0, 1, 4, 2, 3, 5)
        ).reshape(T, KD, 2, 128, 2 * HB)           # [T, KD, half, 128, 2*HB]
        in_maps.append({"x": xc, "w1": w1p, "w2": w2p, "w3": w3p, "bias": bc})

    trace = bool(os.environ.get("SNN_TRACE"))
    last_results = run_bass_kernel_spmd(nc, in_maps, core_ids=list(range(NCORES)),
                                        trace=trace)
    outp = np.empty((B, O), dtype=np.float32)
    for c in range(NCORES):
        outp[c * BC:(c + 1) * BC, :] = last_results.results[c]["out"].T
    return outp
